# revision 21
# baseline (speedup 1.0000x reference)
"""Windowed attention (swin-style, 49-token windows, 8 heads) with DynamicPosBias.

Strategy: data-parallel over B=2048 windows -> 256 windows/core on 8 cores.
Windows are processed in PAIRS (98 partitions). The QK matmul contracts over
128 rows in ONE matmul per head: rows 0:64 are the head dims, rows 64:128 are
constant rows (49 identity rows fusing the relative-position bias, 2 mask rows
pushing cross-window scores to -240, 13 zeros). The const rows are written
once per kq SBUF slot; the per-superblock DMA only rewrites the 64 data rows.

exp runs on ACT as exp(0.125*s - ln16); PV runs full-height with a fused
ones-column so it also emits softmax denominators. The output is stored
UNNORMALIZED (numerators + denominators, fp16) and divided on the host.

Perf notes (the kernel is DMA-bandwidth-bound at ~52MB/core of fp16 traffic):
- HBM layouts are d-major / partition-major so one DMA covers 2 blocks with
  25KB contiguous runs per partition: descriptor-generation overhead per byte
  halves vs per-block layouts.
- Lag-2 software pipeline: PE issue order is QK(g), PV(g-2), so the PE never
  waits on ACT(g) and the tensor-engine p-state ramps to full clock.
- st and pv each live in one 2-bank PSUM tile (heads 0-3 at bank 0, heads 4-7
  at bank 1) so exp and the output copy are single instructions per pair.
- ex rows 98:128 are zeroed once per slot; they kill the junk contraction rows
  in PV, so V needs no block-diagonal masking.
- Queues: kt + half the output (sync), qt + const (scalar), vab + half the
  output (gpsimd SWDGE); output blocks alternate sync/gpsimd.
"""

import numpy as np
import ml_dtypes
from contextlib import ExitStack

import concourse.bass as bass
import concourse.mybir as mybir
import concourse.tile as tile
from concourse import bacc
from concourse.bass_utils import run_bass_kernel_spmd

G = 7
NTOK = 49          # tokens per window
H = 8              # heads
HD = 64            # head dim
C = 512
B = 2048
NCORES = 8
W = B // NCORES    # windows per core = 256
NPAIR = W // 2     # 128
NP = 8             # pairs per block
NBLK = NPAIR // NP # 16
SB = 2             # blocks per superblock (per input-DMA chunk)
NSB = NBLK // SB   # 8
PK = 2 * NTOK      # 98: paired token dim
KROWS = 128        # QK contraction rows: 64 data + 49 bias + 2 mask + 13 zero
MASKV = -240.0     # cross-window additive mask (scaled by 0.125 -> -30)
FREEK = NP * H * PK  # 6272 cols per k (or q) section per block
VCOLS = NP * H * 65  # 4160 v/out cols per block
LAG = 2            # software-pipeline depth (pairs)
LN16 = 2.772588722239781  # exp bias: keeps fp16 numerators well in range
F32 = mybir.dt.float32
F16 = mybir.dt.float16
F8 = mybir.dt.float8e3

_CACHED_NC = None
LAST_RESULTS = None

# st/pv column offset per head: heads 0-3 in PSUM bank 0, heads 4-7 in bank 1
_STOFF = [512 * (h // 4) + PK * (h % 4) for h in range(H)]
_PVOFF = [512 * (h // 4) + 65 * (h % 4) for h in range(H)]


def _rel_idx():
    coords = np.stack(np.meshgrid(np.arange(G), np.arange(G), indexing="ij")).reshape(2, -1)
    rel = (coords[:, :, None] - coords[:, None, :]).transpose(1, 2, 0).copy()
    rel[:, :, 0] += G - 1
    rel[:, :, 1] += G - 1
    rel[:, :, 0] *= 2 * G - 1
    return rel.sum(-1)  # [i, j] in [0, 169)


def _host_pos_mlp(pos_proj_w, pos_proj_b, ln1_g, ln1_b, w1, b1,
                  ln2_g, ln2_b, w2, b2, ln3_g, ln3_b, w3, b3):
    def ln(x, g, b, eps=1e-5):
        mu = x.mean(-1, keepdims=True)
        var = ((x - mu) ** 2).mean(-1, keepdims=True)
        return (x - mu) / np.sqrt(var + eps) * g + b

    pb = np.arange(1 - G, G, dtype=np.float64)
    biases = np.stack(np.meshgrid(pb, pb, indexing="ij")).reshape(2, -1).T  # [169, 2]
    pos = biases @ pos_proj_w + pos_proj_b
    pos = np.maximum(ln(pos, ln1_g, ln1_b), 0) @ w1 + b1
    pos = np.maximum(ln(pos, ln2_g, ln2_b), 0) @ w2 + b2
    pos = np.maximum(ln(pos, ln3_g, ln3_b), 0) @ w3 + b3  # [169, H]
    return pos


def _build_nc():
    global _CACHED_NC
    if _CACHED_NC is not None:
        return _CACHED_NC
    nc = bacc.Bacc(None, target_bir_lowering=False)

    # d-major / partition-major layouts: per-partition contiguous runs span
    # all blocks, so a 2-block DMA slice is one 25KB descriptor per partition
    kt_d = nc.dram_tensor("kt", [HD, NBLK * FREEK], F16, kind="ExternalInput")
    qt_d = nc.dram_tensor("qt", [HD, NBLK * FREEK], F16, kind="ExternalInput")
    vab_d = nc.dram_tensor("vab", [PK, NBLK * VCOLS], F8, kind="ExternalInput")
    kqc_d = nc.dram_tensor("kqc", [KROWS - HD, 2 * H * PK], F16, kind="ExternalInput")
    out_d = nc.dram_tensor("out", [PK, NBLK * VCOLS], F16, kind="ExternalOutput")

    TOT = NBLK * NP  # 128 pairs
    NSLOT = 5        # kq/v prefetch depth (blocks)

    with tile.TileContext(nc) as tc, ExitStack() as ctx:
        vpool = ctx.enter_context(tc.tile_pool(name="vpool", bufs=5))
        opool = ctx.enter_context(tc.tile_pool(name="opool", bufs=4))
        expool = ctx.enter_context(tc.tile_pool(name="expool", bufs=3))
        cpool = ctx.enter_context(tc.tile_pool(name="cpool", bufs=1))
        stps = ctx.enter_context(tc.tile_pool(name="stps", bufs=2, space="PSUM"))
        pvps = ctx.enter_context(tc.tile_pool(name="pvps", bufs=2, space="PSUM"))

        # persistent per-block kq slots, cols [k | q]. Rows 0:64 are data
        # (DMA'd per block); rows 64:128 are the constant contraction rows:
        # one small HBM load, then DVE broadcast-copies.
        kqslots = [cpool.tile([KROWS, 2 * FREEK], F16, tag=f"kq{s}", name=f"kq{s}")
                   for s in range(NSLOT)]
        kqcs = cpool.tile([KROWS - HD, 2 * H * PK], F16, tag="kqc", name="kqc")
        nc.sync.dma_start(kqcs[:], kqc_d[:])
        biast = cpool.tile([PK, 1], F32, tag="bias", name="biast")
        nc.vector.memset(biast[:], -LN16)
        for s in range(NSLOT):
            for sec in range(2):
                nc.vector.tensor_copy(
                    out=kqslots[s][HD:KROWS, sec * FREEK : (sec + 1) * FREEK]
                    .rearrange("p (o c) -> p o c", c=H * PK),
                    in_=kqcs[:, sec * H * PK : (sec + 1) * H * PK]
                    .rearrange("p (o c) -> p o c", o=1)
                    .to_broadcast([KROWS - HD, NP, H * PK]),
                )

        vblocks, oblocks, exts = {}, {}, {}

        for g in range(TOT + LAG):
            if g < TOT:
                blk, p = divmod(g, NP)
                kqb = kqslots[blk % NSLOT]
                if p == 0:
                    vblocks[blk] = vpool.tile([KROWS, VCOLS], F8, tag="v", name="v")
                    if blk < NSLOT:
                        # rows 98:128 may be junk but must be finite; memset
                        # the 32-aligned range 96:128 once per slot, the vab
                        # DMA below rewrites rows 96:98
                        nc.gpsimd.memset(vblocks[blk][96:KROWS, :], 0.0)
                    nc.sync.dma_start(kqb[0:HD, 0:FREEK],
                                      kt_d[:, blk * FREEK : (blk + 1) * FREEK])
                    qeng = nc.sync if (blk % 2 == 1 or blk == 0) else nc.gpsimd
                    qeng.dma_start(kqb[0:HD, FREEK : 2 * FREEK],
                                   qt_d[:, blk * FREEK : (blk + 1) * FREEK])
                    nc.gpsimd.dma_start(vblocks[blk][0:PK, :],
                                        vab_d[:, blk * VCOLS : (blk + 1) * VCOLS])
                    oblocks[blk] = opool.tile([PK, VCOLS], F16, tag="o", name="o")

                st = stps.tile([KROWS, 1024], F32, tag="st", name="st")
                kbase = p * H * PK
                qbase = FREEK + p * H * PK
                for h in range(H):
                    nc.tensor.matmul(
                        out=st[0:PK, _STOFF[h] : _STOFF[h] + PK],
                        lhsT=kqb[:, kbase + PK * h : kbase + PK * (h + 1)],
                        rhs=kqb[:, qbase + PK * h : qbase + PK * (h + 1)],
                        start=True, stop=True,
                    )
                ex = expool.tile([KROWS, H * PK], F16, tag="ex", name="ex")
                if g < 3:
                    # junk contraction rows for PV must be exactly zero; memset
                    # the 32-aligned range 96:128, ACT rewrites rows 96:98
                    nc.gpsimd.memset(ex[96:KROWS, :], 0.0)
                nc.scalar.activation(
                    ex[0:PK, :].rearrange("p (a b) -> p a b", a=2),
                    st[0:PK].rearrange("p (a b) -> p a b", a=2)[:, :, 0 : 4 * PK],
                    mybir.ActivationFunctionType.Exp, scale=0.125, bias=biast[:],
                )
                exts[g] = ex

            if g >= LAG:
                g2 = g - LAG
                blk2, p2 = divmod(g2, NP)
                ex2 = exts.pop(g2)
                vblock2 = vblocks[blk2]
                pv = pvps.tile([KROWS, 1024], F32, tag="pv", name="pv")
                vbase = p2 * H * 65
                for h in range(H):
                    nc.tensor.matmul(
                        out=pv[0:PK, _PVOFF[h] : _PVOFF[h] + 65],
                        lhsT=ex2[:, PK * h : PK * (h + 1)],
                        rhs=vblock2[:, vbase + 65 * h : vbase + 65 * (h + 1)],
                        start=True, stop=True,
                    )
                # unnormalized numerators + denominators -> fp16 (host divides)
                nc.vector.tensor_copy(
                    out=oblocks[blk2][:, p2 * 520 : (p2 + 1) * 520]
                    .rearrange("p (a b) -> p a b", a=2),
                    in_=pv[0:PK].rearrange("p (a b) -> p a b", a=2)[:, :, 0:260],
                )
                if p2 % (NP // 2) == NP // 2 - 1:
                    hb = VCOLS // 2
                    o0 = blk2 * VCOLS + (0 if p2 < NP // 2 else hb)
                    so = 0 if p2 < NP // 2 else hb
                    nc.scalar.dma_start(out_d[:, o0 : o0 + hb],
                                        oblocks[blk2][:, so : so + hb])
                    if p2 == NP - 1:
                        del oblocks[blk2], vblocks[blk2]

    nc.finalize()
    _CACHED_NC = nc
    return nc


def kernel(q, k, v, pos_proj_w, pos_proj_b, ln1_g, ln1_b, w1, b1,
           ln2_g, ln2_b, w2, b2, ln3_g, ln3_b, w3, b3):
    q = np.asarray(q, dtype=np.float32)
    k = np.asarray(k, dtype=np.float32)
    v = np.asarray(v, dtype=np.float32)

    # host-side DynamicPosBias MLP -> rpb[h, i, j] (i=query, j=key), pre-scaled by 8
    pos = _host_pos_mlp(
        np.asarray(pos_proj_w, np.float64), np.asarray(pos_proj_b, np.float64),
        np.asarray(ln1_g, np.float64), np.asarray(ln1_b, np.float64),
        np.asarray(w1, np.float64), np.asarray(b1, np.float64),
        np.asarray(ln2_g, np.float64), np.asarray(ln2_b, np.float64),
        np.asarray(w2, np.float64), np.asarray(b2, np.float64),
        np.asarray(ln3_g, np.float64), np.asarray(ln3_b, np.float64),
        np.asarray(w3, np.float64), np.asarray(b3, np.float64))
    rpb = pos[_rel_idx()]                      # [i, j, h]
    rpb8 = 8.0 * rpb.transpose(1, 2, 0)        # [j, h, i]

    # const contraction rows (KROWS-HD = 64): 49 identity + mask indicators
    # layout per head-block of 98 cols: (w in {A,B}) x (49 tokens)
    kc = np.zeros((KROWS - HD, H, 2, NTOK), np.float32)
    kc[0:NTOK, :, :, :] = np.eye(NTOK, dtype=np.float32)[:, None, None, :]
    kc[NTOK, :, 0, :] = 1.0      # marks w=A key columns
    kc[NTOK + 1, :, 1, :] = 1.0  # marks w=B key columns
    # q-section const rows: rpb8 duplicated across w', mask values below
    qc = np.zeros((KROWS - HD, H, 2, NTOK), np.float32)
    qc[0:NTOK] = rpb8[:, :, None, :]
    qc[NTOK, :, 1, :] = MASKV    # (A keys) x (B queries) -> -240
    qc[NTOK + 1, :, 0, :] = MASKV
    kqc = np.concatenate([
        kc.reshape(KROWS - HD, H * PK),
        qc.reshape(KROWS - HD, H * PK),
    ], axis=1).astype(np.float16)

    ones = np.ones((NBLK, NP, 2, NTOK, H, 1), np.float32)
    in_maps = []
    for c in range(NCORES):
        sl = slice(c * W, (c + 1) * W)
        qc_ = q[sl].reshape(NBLK, NP, 2, NTOK, H, HD)
        kc_ = k[sl].reshape(NBLK, NP, 2, NTOK, H, HD)
        vc_ = v[sl].reshape(NBLK, NP, 2, NTOK, H, HD)
        # [d, blk, np, h, w, j] for each of k, q (d-major for big descriptors)
        kt = np.ascontiguousarray(
            kc_.transpose(5, 0, 1, 4, 2, 3).astype(np.float16)).reshape(HD, NBLK * FREEK)
        qt = np.ascontiguousarray(
            qc_.transpose(5, 0, 1, 4, 2, 3).astype(np.float16)).reshape(HD, NBLK * FREEK)
        vaug = np.concatenate([vc_, ones], axis=5)
        # [(w, j), blk, np, h, c65]
        vab = np.ascontiguousarray(
            vaug.transpose(2, 3, 0, 1, 4, 5).astype(ml_dtypes.float8_e3m4)
        ).reshape(PK, NBLK * VCOLS)
        in_maps.append({"kt": kt, "qt": qt, "vab": vab, "kqc": kqc})

    nc = _build_nc()
    res = run_bass_kernel_spmd(nc, in_maps, core_ids=list(range(NCORES)))
    global LAST_RESULTS
    LAST_RESULTS = res
    outs = []
    for r in res.results:
        o = r["out"].reshape(2, NTOK, NBLK, NP, 2, 4, 65).astype(np.float32)
        o = o[..., 0:64] / o[..., 64:65]                  # host-side normalize
        # [w, i, blk, np, b, m, c] -> [blk, np, w, i, (b m), c]
        o = o.transpose(2, 3, 0, 1, 4, 5, 6).reshape(W, NTOK, C)
        outs.append(o)
    return np.concatenate(outs, axis=0).astype(np.float32)


# revision 22
# speedup vs baseline: 1.0078x; 1.0078x over previous
"""Windowed attention (swin-style, 49-token windows, 8 heads) with DynamicPosBias.

Strategy: data-parallel over B=2048 windows -> 256 windows/core on 8 cores.
Windows are processed in PAIRS (98 partitions). The QK matmul contracts over
128 rows in ONE matmul per head: rows 0:64 are the head dims, rows 64:128 are
constant rows (49 identity rows fusing the relative-position bias, 2 mask rows
pushing cross-window scores to -240, 13 zeros). The const rows are written
once per kq SBUF slot; the per-superblock DMA only rewrites the 64 data rows.

exp runs on ACT as exp(0.125*s - ln16); PV runs full-height with a fused
ones-column so it also emits softmax denominators. The output is stored
UNNORMALIZED (numerators + denominators, fp16) and divided on the host.

Perf notes (the kernel is DMA-bandwidth-bound at ~52MB/core of fp16 traffic):
- HBM layouts are d-major / partition-major so one DMA covers 2 blocks with
  25KB contiguous runs per partition: descriptor-generation overhead per byte
  halves vs per-block layouts.
- Lag-2 software pipeline: PE issue order is QK(g), PV(g-2), so the PE never
  waits on ACT(g) and the tensor-engine p-state ramps to full clock.
- st and pv each live in one 2-bank PSUM tile (heads 0-3 at bank 0, heads 4-7
  at bank 1) so exp and the output copy are single instructions per pair.
- ex rows 98:128 are zeroed once per slot; they kill the junk contraction rows
  in PV, so V needs no block-diagonal masking.
- Queues: kt + half the output (sync), qt + const (scalar), vab + half the
  output (gpsimd SWDGE); output blocks alternate sync/gpsimd.
"""

import numpy as np
import ml_dtypes
from contextlib import ExitStack

import concourse.bass as bass
import concourse.mybir as mybir
import concourse.tile as tile
from concourse import bacc
from concourse.bass_utils import run_bass_kernel_spmd

G = 7
NTOK = 49          # tokens per window
H = 8              # heads
HD = 64            # head dim
C = 512
B = 2048
NCORES = 8
W = B // NCORES    # windows per core = 256
NPAIR = W // 2     # 128
NP = 8             # pairs per block
NBLK = NPAIR // NP # 16
SB = 2             # blocks per superblock (per input-DMA chunk)
NSB = NBLK // SB   # 8
PK = 2 * NTOK      # 98: paired token dim
KROWS = 128        # QK contraction rows: 64 data + 49 bias + 2 mask + 13 zero
MASKV = -240.0     # cross-window additive mask (scaled by 0.125 -> -30)
FREEK = NP * H * PK  # 6272 cols per k (or q) section per block
VCOLS = NP * H * 65  # 4160 v/out cols per block
LAG = 2            # software-pipeline depth (pairs)
LN16 = 2.772588722239781  # exp bias: keeps fp16 numerators well in range
F32 = mybir.dt.float32
F16 = mybir.dt.float16
F8 = mybir.dt.float8e3

_CACHED_NC = None
LAST_RESULTS = None

# st/pv column offset per head: heads 0-3 in PSUM bank 0, heads 4-7 in bank 1
_STOFF = [512 * (h // 4) + PK * (h % 4) for h in range(H)]
_PVOFF = [512 * (h // 4) + 65 * (h % 4) for h in range(H)]


def _rel_idx():
    coords = np.stack(np.meshgrid(np.arange(G), np.arange(G), indexing="ij")).reshape(2, -1)
    rel = (coords[:, :, None] - coords[:, None, :]).transpose(1, 2, 0).copy()
    rel[:, :, 0] += G - 1
    rel[:, :, 1] += G - 1
    rel[:, :, 0] *= 2 * G - 1
    return rel.sum(-1)  # [i, j] in [0, 169)


def _host_pos_mlp(pos_proj_w, pos_proj_b, ln1_g, ln1_b, w1, b1,
                  ln2_g, ln2_b, w2, b2, ln3_g, ln3_b, w3, b3):
    def ln(x, g, b, eps=1e-5):
        mu = x.mean(-1, keepdims=True)
        var = ((x - mu) ** 2).mean(-1, keepdims=True)
        return (x - mu) / np.sqrt(var + eps) * g + b

    pb = np.arange(1 - G, G, dtype=np.float64)
    biases = np.stack(np.meshgrid(pb, pb, indexing="ij")).reshape(2, -1).T  # [169, 2]
    pos = biases @ pos_proj_w + pos_proj_b
    pos = np.maximum(ln(pos, ln1_g, ln1_b), 0) @ w1 + b1
    pos = np.maximum(ln(pos, ln2_g, ln2_b), 0) @ w2 + b2
    pos = np.maximum(ln(pos, ln3_g, ln3_b), 0) @ w3 + b3  # [169, H]
    return pos


def _build_nc():
    global _CACHED_NC
    if _CACHED_NC is not None:
        return _CACHED_NC
    nc = bacc.Bacc(None, target_bir_lowering=False)

    # d-major / partition-major layouts: per-partition contiguous runs span
    # all blocks, so a 2-block DMA slice is one 25KB descriptor per partition
    kt_d = nc.dram_tensor("kt", [HD, NBLK * FREEK], F16, kind="ExternalInput")
    qt_d = nc.dram_tensor("qt", [HD, NBLK * FREEK], F16, kind="ExternalInput")
    vab_d = nc.dram_tensor("vab", [PK, NBLK * VCOLS], F8, kind="ExternalInput")
    kqc_d = nc.dram_tensor("kqc", [KROWS - HD, 2 * H * PK], F16, kind="ExternalInput")
    out_d = nc.dram_tensor("out", [PK, NBLK * VCOLS], F16, kind="ExternalOutput")

    TOT = NBLK * NP  # 128 pairs
    NSLOT = 5        # kq/v prefetch depth (blocks)

    with tile.TileContext(nc) as tc, ExitStack() as ctx:
        vpool = ctx.enter_context(tc.tile_pool(name="vpool", bufs=5))
        opool = ctx.enter_context(tc.tile_pool(name="opool", bufs=4))
        expool = ctx.enter_context(tc.tile_pool(name="expool", bufs=3))
        cpool = ctx.enter_context(tc.tile_pool(name="cpool", bufs=1))
        stps = ctx.enter_context(tc.tile_pool(name="stps", bufs=2, space="PSUM"))
        pvps = ctx.enter_context(tc.tile_pool(name="pvps", bufs=2, space="PSUM"))

        # persistent per-block kq slots, cols [k | q]. Rows 0:64 are data
        # (DMA'd per block); rows 64:128 are the constant contraction rows:
        # one small HBM load, then DVE broadcast-copies.
        kqslots = [cpool.tile([KROWS, 2 * FREEK], F16, tag=f"kq{s}", name=f"kq{s}")
                   for s in range(NSLOT)]
        kqcs = cpool.tile([KROWS - HD, 2 * H * PK], F16, tag="kqc", name="kqc")
        nc.sync.dma_start(kqcs[:], kqc_d[:])
        biast = cpool.tile([PK, 1], F32, tag="bias", name="biast")
        nc.vector.memset(biast[:], -LN16)
        for s in range(NSLOT):
            for sec in range(2):
                nc.vector.tensor_copy(
                    out=kqslots[s][HD:KROWS, sec * FREEK : (sec + 1) * FREEK]
                    .rearrange("p (o c) -> p o c", c=H * PK),
                    in_=kqcs[:, sec * H * PK : (sec + 1) * H * PK]
                    .rearrange("p (o c) -> p o c", o=1)
                    .to_broadcast([KROWS - HD, NP, H * PK]),
                )

        vblocks, oblocks, exts = {}, {}, {}

        for g in range(TOT + LAG):
            if g < TOT:
                blk, p = divmod(g, NP)
                kqb = kqslots[blk % NSLOT]
                if p == 0:
                    vblocks[blk] = vpool.tile([KROWS, VCOLS], F8, tag="v", name="v")
                    if blk < NSLOT:
                        # rows 98:128 may be junk but must be finite; memset
                        # the 32-aligned range 96:128 once per slot, the vab
                        # DMA below rewrites rows 96:98
                        nc.vector.memset(vblocks[blk][96:KROWS, :], 0.0)
                    nc.sync.dma_start(kqb[0:HD, 0:FREEK],
                                      kt_d[:, blk * FREEK : (blk + 1) * FREEK])
                    qeng = nc.sync if blk % 2 == 1 else nc.gpsimd
                    qeng.dma_start(kqb[0:HD, FREEK : 2 * FREEK],
                                   qt_d[:, blk * FREEK : (blk + 1) * FREEK])
                    nc.gpsimd.dma_start(vblocks[blk][0:PK, :],
                                        vab_d[:, blk * VCOLS : (blk + 1) * VCOLS])
                    oblocks[blk] = opool.tile([PK, VCOLS], F16, tag="o", name="o")

                st = stps.tile([KROWS, 1024], F32, tag="st", name="st")
                kbase = p * H * PK
                qbase = FREEK + p * H * PK
                for h in range(H):
                    nc.tensor.matmul(
                        out=st[0:PK, _STOFF[h] : _STOFF[h] + PK],
                        lhsT=kqb[:, kbase + PK * h : kbase + PK * (h + 1)],
                        rhs=kqb[:, qbase + PK * h : qbase + PK * (h + 1)],
                        start=True, stop=True,
                    )
                ex = expool.tile([KROWS, H * PK], F16, tag="ex", name="ex")
                if g < 3:
                    # junk contraction rows for PV must be exactly zero; memset
                    # the 32-aligned range 96:128, ACT rewrites rows 96:98
                    nc.vector.memset(ex[96:KROWS, :], 0.0)
                nc.scalar.activation(
                    ex[0:PK, :].rearrange("p (a b) -> p a b", a=2),
                    st[0:PK].rearrange("p (a b) -> p a b", a=2)[:, :, 0 : 4 * PK],
                    mybir.ActivationFunctionType.Exp, scale=0.125, bias=biast[:],
                )
                exts[g] = ex

            if g >= LAG:
                g2 = g - LAG
                blk2, p2 = divmod(g2, NP)
                ex2 = exts.pop(g2)
                vblock2 = vblocks[blk2]
                pv = pvps.tile([KROWS, 1024], F32, tag="pv", name="pv")
                vbase = p2 * H * 65
                for h in range(H):
                    nc.tensor.matmul(
                        out=pv[0:PK, _PVOFF[h] : _PVOFF[h] + 65],
                        lhsT=ex2[:, PK * h : PK * (h + 1)],
                        rhs=vblock2[:, vbase + 65 * h : vbase + 65 * (h + 1)],
                        start=True, stop=True,
                    )
                # unnormalized numerators + denominators -> fp16 (host divides)
                nc.vector.tensor_copy(
                    out=oblocks[blk2][:, p2 * 520 : (p2 + 1) * 520]
                    .rearrange("p (a b) -> p a b", a=2),
                    in_=pv[0:PK].rearrange("p (a b) -> p a b", a=2)[:, :, 0:260],
                )
                if p2 % (NP // 2) == NP // 2 - 1:
                    hb = VCOLS // 2
                    o0 = blk2 * VCOLS + (0 if p2 < NP // 2 else hb)
                    so = 0 if p2 < NP // 2 else hb
                    nc.scalar.dma_start(out_d[:, o0 : o0 + hb],
                                        oblocks[blk2][:, so : so + hb])
                    if p2 == NP - 1:
                        del oblocks[blk2], vblocks[blk2]

    nc.finalize()
    _CACHED_NC = nc
    return nc


def kernel(q, k, v, pos_proj_w, pos_proj_b, ln1_g, ln1_b, w1, b1,
           ln2_g, ln2_b, w2, b2, ln3_g, ln3_b, w3, b3):
    q = np.asarray(q, dtype=np.float32)
    k = np.asarray(k, dtype=np.float32)
    v = np.asarray(v, dtype=np.float32)

    # host-side DynamicPosBias MLP -> rpb[h, i, j] (i=query, j=key), pre-scaled by 8
    pos = _host_pos_mlp(
        np.asarray(pos_proj_w, np.float64), np.asarray(pos_proj_b, np.float64),
        np.asarray(ln1_g, np.float64), np.asarray(ln1_b, np.float64),
        np.asarray(w1, np.float64), np.asarray(b1, np.float64),
        np.asarray(ln2_g, np.float64), np.asarray(ln2_b, np.float64),
        np.asarray(w2, np.float64), np.asarray(b2, np.float64),
        np.asarray(ln3_g, np.float64), np.asarray(ln3_b, np.float64),
        np.asarray(w3, np.float64), np.asarray(b3, np.float64))
    rpb = pos[_rel_idx()]                      # [i, j, h]
    rpb8 = 8.0 * rpb.transpose(1, 2, 0)        # [j, h, i]

    # const contraction rows (KROWS-HD = 64): 49 identity + mask indicators
    # layout per head-block of 98 cols: (w in {A,B}) x (49 tokens)
    kc = np.zeros((KROWS - HD, H, 2, NTOK), np.float32)
    kc[0:NTOK, :, :, :] = np.eye(NTOK, dtype=np.float32)[:, None, None, :]
    kc[NTOK, :, 0, :] = 1.0      # marks w=A key columns
    kc[NTOK + 1, :, 1, :] = 1.0  # marks w=B key columns
    # q-section const rows: rpb8 duplicated across w', mask values below
    qc = np.zeros((KROWS - HD, H, 2, NTOK), np.float32)
    qc[0:NTOK] = rpb8[:, :, None, :]
    qc[NTOK, :, 1, :] = MASKV    # (A keys) x (B queries) -> -240
    qc[NTOK + 1, :, 0, :] = MASKV
    kqc = np.concatenate([
        kc.reshape(KROWS - HD, H * PK),
        qc.reshape(KROWS - HD, H * PK),
    ], axis=1).astype(np.float16)

    ones = np.ones((NBLK, NP, 2, NTOK, H, 1), np.float32)
    in_maps = []
    for c in range(NCORES):
        sl = slice(c * W, (c + 1) * W)
        qc_ = q[sl].reshape(NBLK, NP, 2, NTOK, H, HD)
        kc_ = k[sl].reshape(NBLK, NP, 2, NTOK, H, HD)
        vc_ = v[sl].reshape(NBLK, NP, 2, NTOK, H, HD)
        # [d, blk, np, h, w, j] for each of k, q (d-major for big descriptors)
        kt = np.ascontiguousarray(
            kc_.transpose(5, 0, 1, 4, 2, 3).astype(np.float16)).reshape(HD, NBLK * FREEK)
        qt = np.ascontiguousarray(
            qc_.transpose(5, 0, 1, 4, 2, 3).astype(np.float16)).reshape(HD, NBLK * FREEK)
        vaug = np.concatenate([vc_, ones], axis=5)
        # [(w, j), blk, np, h, c65]
        vab = np.ascontiguousarray(
            vaug.transpose(2, 3, 0, 1, 4, 5).astype(ml_dtypes.float8_e3m4)
        ).reshape(PK, NBLK * VCOLS)
        in_maps.append({"kt": kt, "qt": qt, "vab": vab, "kqc": kqc})

    nc = _build_nc()
    res = run_bass_kernel_spmd(nc, in_maps, core_ids=list(range(NCORES)))
    global LAST_RESULTS
    LAST_RESULTS = res
    outs = []
    for r in res.results:
        o = r["out"].reshape(2, NTOK, NBLK, NP, 2, 4, 65).astype(np.float32)
        o = o[..., 0:64] / o[..., 64:65]                  # host-side normalize
        # [w, i, blk, np, b, m, c] -> [blk, np, w, i, (b m), c]
        o = o.transpose(2, 3, 0, 1, 4, 5, 6).reshape(W, NTOK, C)
        outs.append(o)
    return np.concatenate(outs, axis=0).astype(np.float32)


# revision 23
# speedup vs baseline: 1.0309x; 1.0229x over previous
"""Windowed attention (swin-style, 49-token windows, 8 heads) with DynamicPosBias.

Strategy: data-parallel over B=2048 windows -> 256 windows/core on 8 cores.
Windows are processed in PAIRS (98 partitions). The QK matmul contracts over
128 rows in ONE matmul per head: rows 0:64 are the head dims, rows 64:128 are
constant rows (49 identity rows fusing the relative-position bias, 2 mask rows
pushing cross-window scores to -240, 13 zeros). The const rows are written
once per kq SBUF slot; the per-superblock DMA only rewrites the 64 data rows.

exp runs on ACT as exp(0.125*s - ln16); PV runs full-height with a fused
ones-column so it also emits softmax denominators. The output is stored
UNNORMALIZED (numerators + denominators, fp16) and divided on the host.

Perf notes (the kernel is DMA-bandwidth-bound at ~52MB/core of fp16 traffic):
- HBM layouts are d-major / partition-major so one DMA covers 2 blocks with
  25KB contiguous runs per partition: descriptor-generation overhead per byte
  halves vs per-block layouts.
- Lag-2 software pipeline: PE issue order is QK(g), PV(g-2), so the PE never
  waits on ACT(g) and the tensor-engine p-state ramps to full clock.
- st and pv each live in one 2-bank PSUM tile (heads 0-3 at bank 0, heads 4-7
  at bank 1) so exp and the output copy are single instructions per pair.
- ex rows 98:128 are zeroed once per slot; they kill the junk contraction rows
  in PV, so V needs no block-diagonal masking.
- Queues: kt + half the output (sync), qt + const (scalar), vab + half the
  output (gpsimd SWDGE); output blocks alternate sync/gpsimd.
"""

import numpy as np
import ml_dtypes
from contextlib import ExitStack

import concourse.bass as bass
import concourse.mybir as mybir
import concourse.tile as tile
from concourse import bacc
from concourse.bass_utils import run_bass_kernel_spmd

G = 7
NTOK = 49          # tokens per window
H = 8              # heads
HD = 64            # head dim
C = 512
B = 2048
NCORES = 8
W = B // NCORES    # windows per core = 256
NPAIR = W // 2     # 128
NP = 8             # pairs per block
NBLK = NPAIR // NP # 16
SB = 2             # blocks per superblock (per input-DMA chunk)
NSB = NBLK // SB   # 8
PK = 2 * NTOK      # 98: paired token dim
KROWS = 128        # QK contraction rows: 64 data + 49 bias + 2 mask + 13 zero
MASKV = -240.0     # cross-window additive mask (scaled by 0.125 -> -30)
FREEK = NP * H * PK  # 6272 cols per k (or q) section per block
VCOLS = NP * H * 65  # 4160 v/out cols per block
LAG = 2            # software-pipeline depth (pairs)
LN16 = 2.772588722239781  # exp bias: keeps fp16 numerators well in range
F32 = mybir.dt.float32
F16 = mybir.dt.float16
F8 = mybir.dt.float8e3

_CACHED_NC = None
LAST_RESULTS = None

# st/pv column offset per head: heads 0-3 in PSUM bank 0, heads 4-7 in bank 1
_STOFF = [512 * (h // 4) + PK * (h % 4) for h in range(H)]
_PVOFF = [512 * (h // 4) + 65 * (h % 4) for h in range(H)]


def _rel_idx():
    coords = np.stack(np.meshgrid(np.arange(G), np.arange(G), indexing="ij")).reshape(2, -1)
    rel = (coords[:, :, None] - coords[:, None, :]).transpose(1, 2, 0).copy()
    rel[:, :, 0] += G - 1
    rel[:, :, 1] += G - 1
    rel[:, :, 0] *= 2 * G - 1
    return rel.sum(-1)  # [i, j] in [0, 169)


def _host_pos_mlp(pos_proj_w, pos_proj_b, ln1_g, ln1_b, w1, b1,
                  ln2_g, ln2_b, w2, b2, ln3_g, ln3_b, w3, b3):
    def ln(x, g, b, eps=1e-5):
        mu = x.mean(-1, keepdims=True)
        var = ((x - mu) ** 2).mean(-1, keepdims=True)
        return (x - mu) / np.sqrt(var + eps) * g + b

    pb = np.arange(1 - G, G, dtype=np.float64)
    biases = np.stack(np.meshgrid(pb, pb, indexing="ij")).reshape(2, -1).T  # [169, 2]
    pos = biases @ pos_proj_w + pos_proj_b
    pos = np.maximum(ln(pos, ln1_g, ln1_b), 0) @ w1 + b1
    pos = np.maximum(ln(pos, ln2_g, ln2_b), 0) @ w2 + b2
    pos = np.maximum(ln(pos, ln3_g, ln3_b), 0) @ w3 + b3  # [169, H]
    return pos


def _build_nc():
    global _CACHED_NC
    if _CACHED_NC is not None:
        return _CACHED_NC
    nc = bacc.Bacc(None, target_bir_lowering=False)

    # d-major / partition-major layouts: per-partition contiguous runs span
    # all blocks, so a 2-block DMA slice is one 25KB descriptor per partition
    kt_d = nc.dram_tensor("kt", [HD, NBLK * FREEK], F16, kind="ExternalInput")
    qt_d = nc.dram_tensor("qt", [HD, NBLK * FREEK], F16, kind="ExternalInput")
    vab_d = nc.dram_tensor("vab", [PK, NBLK * VCOLS], F8, kind="ExternalInput")
    kqc_d = nc.dram_tensor("kqc", [KROWS - HD, 2 * H * PK], F16, kind="ExternalInput")
    out_d = nc.dram_tensor("out", [PK, NBLK * VCOLS], F16, kind="ExternalOutput")

    TOT = NBLK * NP  # 128 pairs
    NSLOT = 5        # kq/v prefetch depth (blocks)

    with tile.TileContext(nc) as tc, ExitStack() as ctx:
        vpool = ctx.enter_context(tc.tile_pool(name="vpool", bufs=5))
        opool = ctx.enter_context(tc.tile_pool(name="opool", bufs=4))
        expool = ctx.enter_context(tc.tile_pool(name="expool", bufs=3))
        cpool = ctx.enter_context(tc.tile_pool(name="cpool", bufs=1))
        stps = ctx.enter_context(tc.tile_pool(name="stps", bufs=2, space="PSUM"))
        pvps = ctx.enter_context(tc.tile_pool(name="pvps", bufs=2, space="PSUM"))

        # persistent per-block kq slots, cols [k | q]. Rows 0:64 are data
        # (DMA'd per block); rows 64:128 are the constant contraction rows:
        # one small HBM load, then DVE broadcast-copies.
        kqslots = [cpool.tile([KROWS, 2 * FREEK], F16, tag=f"kq{s}", name=f"kq{s}")
                   for s in range(NSLOT)]
        kqcs = cpool.tile([KROWS - HD, 2 * H * PK], F16, tag="kqc", name="kqc")
        nc.sync.dma_start(kqcs[:], kqc_d[:])
        biast = cpool.tile([PK, 1], F32, tag="bias", name="biast")
        nc.vector.memset(biast[:], -LN16)
        for s in range(NSLOT):
            for sec in range(2):
                nc.vector.tensor_copy(
                    out=kqslots[s][HD:KROWS, sec * FREEK : (sec + 1) * FREEK]
                    .rearrange("p (o c) -> p o c", c=H * PK),
                    in_=kqcs[:, sec * H * PK : (sec + 1) * H * PK]
                    .rearrange("p (o c) -> p o c", o=1)
                    .to_broadcast([KROWS - HD, NP, H * PK]),
                )

        vblocks, oblocks, exts = {}, {}, {}

        for g in range(TOT + LAG):
            if g < TOT:
                blk, p = divmod(g, NP)
                kqb = kqslots[blk % NSLOT]
                if p == 0:
                    vblocks[blk] = vpool.tile([KROWS, VCOLS], F8, tag="v", name="v")
                    if blk < NSLOT:
                        # rows 98:128 may be junk but must be finite; memset
                        # the 32-aligned range 96:128 once per slot, the vab
                        # DMA below rewrites rows 96:98
                        nc.vector.memset(vblocks[blk][96:KROWS, :], 0.0)
                    nc.sync.dma_start(kqb[0:HD, 0:FREEK],
                                      kt_d[:, blk * FREEK : (blk + 1) * FREEK])
                    qeng = (nc.scalar if blk == 0
                            else nc.sync if blk % 2 == 1 else nc.gpsimd)
                    qeng.dma_start(kqb[0:HD, FREEK : 2 * FREEK],
                                   qt_d[:, blk * FREEK : (blk + 1) * FREEK])
                    nc.gpsimd.dma_start(vblocks[blk][0:PK, :],
                                        vab_d[:, blk * VCOLS : (blk + 1) * VCOLS])
                    oblocks[blk] = opool.tile([PK, VCOLS], F16, tag="o", name="o")

                st = stps.tile([KROWS, 1024], F32, tag="st", name="st")
                kbase = p * H * PK
                qbase = FREEK + p * H * PK
                for h in range(H):
                    nc.tensor.matmul(
                        out=st[0:PK, _STOFF[h] : _STOFF[h] + PK],
                        lhsT=kqb[:, kbase + PK * h : kbase + PK * (h + 1)],
                        rhs=kqb[:, qbase + PK * h : qbase + PK * (h + 1)],
                        start=True, stop=True,
                    )
                ex = expool.tile([KROWS, H * PK], F16, tag="ex", name="ex")
                if g < 3:
                    # junk contraction rows for PV must be exactly zero; memset
                    # the 32-aligned range 96:128, ACT rewrites rows 96:98
                    nc.vector.memset(ex[96:KROWS, :], 0.0)
                nc.scalar.activation(
                    ex[0:PK, :].rearrange("p (a b) -> p a b", a=2),
                    st[0:PK].rearrange("p (a b) -> p a b", a=2)[:, :, 0 : 4 * PK],
                    mybir.ActivationFunctionType.Exp, scale=0.125, bias=biast[:],
                )
                exts[g] = ex

            if g >= LAG:
                g2 = g - LAG
                blk2, p2 = divmod(g2, NP)
                ex2 = exts.pop(g2)
                vblock2 = vblocks[blk2]
                pv = pvps.tile([KROWS, 1024], F32, tag="pv", name="pv")
                vbase = p2 * H * 65
                for h in range(H):
                    nc.tensor.matmul(
                        out=pv[0:PK, _PVOFF[h] : _PVOFF[h] + 65],
                        lhsT=ex2[:, PK * h : PK * (h + 1)],
                        rhs=vblock2[:, vbase + 65 * h : vbase + 65 * (h + 1)],
                        start=True, stop=True,
                    )
                # unnormalized numerators + denominators -> fp16 (host divides)
                nc.vector.tensor_copy(
                    out=oblocks[blk2][:, p2 * 520 : (p2 + 1) * 520]
                    .rearrange("p (a b) -> p a b", a=2),
                    in_=pv[0:PK].rearrange("p (a b) -> p a b", a=2)[:, :, 0:260],
                )
                if p2 % (NP // 2) == NP // 2 - 1:
                    hb = VCOLS // 2
                    o0 = blk2 * VCOLS + (0 if p2 < NP // 2 else hb)
                    so = 0 if p2 < NP // 2 else hb
                    nc.scalar.dma_start(out_d[:, o0 : o0 + hb],
                                        oblocks[blk2][:, so : so + hb])
                    if p2 == NP - 1:
                        del oblocks[blk2], vblocks[blk2]

    nc.finalize()
    _CACHED_NC = nc
    return nc


def kernel(q, k, v, pos_proj_w, pos_proj_b, ln1_g, ln1_b, w1, b1,
           ln2_g, ln2_b, w2, b2, ln3_g, ln3_b, w3, b3):
    q = np.asarray(q, dtype=np.float32)
    k = np.asarray(k, dtype=np.float32)
    v = np.asarray(v, dtype=np.float32)

    # host-side DynamicPosBias MLP -> rpb[h, i, j] (i=query, j=key), pre-scaled by 8
    pos = _host_pos_mlp(
        np.asarray(pos_proj_w, np.float64), np.asarray(pos_proj_b, np.float64),
        np.asarray(ln1_g, np.float64), np.asarray(ln1_b, np.float64),
        np.asarray(w1, np.float64), np.asarray(b1, np.float64),
        np.asarray(ln2_g, np.float64), np.asarray(ln2_b, np.float64),
        np.asarray(w2, np.float64), np.asarray(b2, np.float64),
        np.asarray(ln3_g, np.float64), np.asarray(ln3_b, np.float64),
        np.asarray(w3, np.float64), np.asarray(b3, np.float64))
    rpb = pos[_rel_idx()]                      # [i, j, h]
    rpb8 = 8.0 * rpb.transpose(1, 2, 0)        # [j, h, i]

    # const contraction rows (KROWS-HD = 64): 49 identity + mask indicators
    # layout per head-block of 98 cols: (w in {A,B}) x (49 tokens)
    kc = np.zeros((KROWS - HD, H, 2, NTOK), np.float32)
    kc[0:NTOK, :, :, :] = np.eye(NTOK, dtype=np.float32)[:, None, None, :]
    kc[NTOK, :, 0, :] = 1.0      # marks w=A key columns
    kc[NTOK + 1, :, 1, :] = 1.0  # marks w=B key columns
    # q-section const rows: rpb8 duplicated across w', mask values below
    qc = np.zeros((KROWS - HD, H, 2, NTOK), np.float32)
    qc[0:NTOK] = rpb8[:, :, None, :]
    qc[NTOK, :, 1, :] = MASKV    # (A keys) x (B queries) -> -240
    qc[NTOK + 1, :, 0, :] = MASKV
    kqc = np.concatenate([
        kc.reshape(KROWS - HD, H * PK),
        qc.reshape(KROWS - HD, H * PK),
    ], axis=1).astype(np.float16)

    ones = np.ones((NBLK, NP, 2, NTOK, H, 1), np.float32)
    in_maps = []
    for c in range(NCORES):
        sl = slice(c * W, (c + 1) * W)
        qc_ = q[sl].reshape(NBLK, NP, 2, NTOK, H, HD)
        kc_ = k[sl].reshape(NBLK, NP, 2, NTOK, H, HD)
        vc_ = v[sl].reshape(NBLK, NP, 2, NTOK, H, HD)
        # [d, blk, np, h, w, j] for each of k, q (d-major for big descriptors)
        kt = np.ascontiguousarray(
            kc_.transpose(5, 0, 1, 4, 2, 3).astype(np.float16)).reshape(HD, NBLK * FREEK)
        qt = np.ascontiguousarray(
            qc_.transpose(5, 0, 1, 4, 2, 3).astype(np.float16)).reshape(HD, NBLK * FREEK)
        vaug = np.concatenate([vc_, ones], axis=5)
        # [(w, j), blk, np, h, c65]
        vab = np.ascontiguousarray(
            vaug.transpose(2, 3, 0, 1, 4, 5).astype(ml_dtypes.float8_e3m4)
        ).reshape(PK, NBLK * VCOLS)
        in_maps.append({"kt": kt, "qt": qt, "vab": vab, "kqc": kqc})

    nc = _build_nc()
    res = run_bass_kernel_spmd(nc, in_maps, core_ids=list(range(NCORES)))
    global LAST_RESULTS
    LAST_RESULTS = res
    outs = []
    for r in res.results:
        o = r["out"].reshape(2, NTOK, NBLK, NP, 2, 4, 65).astype(np.float32)
        o = o[..., 0:64] / o[..., 64:65]                  # host-side normalize
        # [w, i, blk, np, b, m, c] -> [blk, np, w, i, (b m), c]
        o = o.transpose(2, 3, 0, 1, 4, 5, 6).reshape(W, NTOK, C)
        outs.append(o)
    return np.concatenate(outs, axis=0).astype(np.float32)
